# revision 1
# baseline (speedup 1.0000x reference)
"""Trainium2 Bass kernel for DiscriminativeLoss (segment_reduce).

Full inputs: embedding [8, 32, 65536] f32, seg_gt [8, 65536] i32 (labels 0..20,
0 = background).  Output: (var_loss, dist_loss, reg_loss) scalars.

Sharding: pure data parallel — batch b -> core b.  Each core computes, for its
sample:
  pass 1 (pixel-on-partition layout): per-label sums[21,32] + counts[21] via
         one-hot matmuls accumulated in PSUM,
  pass 2 (channel-on-partition layout): per-pixel squared distance to own
         centroid via accumulated (I | -M) matmuls, hinge, and the w-weighted
         global reduction where w_l = present_l / counts_l.
The tiny 21x21 centroid pairwise loss and final scalar assembly run on host
from the per-core [84,129] segment-sum matrix and [128] partial var sums.
"""

import os
import sys
from contextlib import ExitStack

import numpy as np

for _p in ("/opt/trn_rl_repo", "/root/.axon_site/_ro/trn_rl_repo"):
    if os.path.isdir(_p) and _p not in sys.path:
        sys.path.insert(0, _p)

import ml_dtypes

import concourse.bass as bass
import concourse.bacc as bacc
import concourse.tile as tile
from concourse import mybir
from concourse.bass_utils import run_bass_kernel_spmd

BF16 = ml_dtypes.bfloat16

B, D, N = 8, 32, 65536
LP = 21          # label slots 0..20 (0 = background)
C = 4            # chunk count (channel-on-partition packing)
NC4 = N // C     # 16384 pixels per chunk
G = 128          # pass-1 tiles (512 px each)
A4 = 4           # pixels per partition per pass-1 tile
T2 = 32          # pass-2 tiles (512 cols each)
DELTA_V = 0.5
DELTA_D = 3.0

# const tensor column offsets (bf16 [128, CST_W])
OFF_IOTA_L = 0            # [128, 672]  l pattern, tiled x8 slabs
OFF_IOTA_COL = 672        # [128, 1]    p % 32
OFF_IDENT = 673           # [128, 128]  identity
OFF_SEL = 801             # [128, 84]   eye(84) selector
OFF_ONES_BD8 = 885        # [128, 256]  8 shifted block-diag ones variants
OFF_MASK = 1141           # [128, 1]    1 for rows c*32+l with 1<=l<=20
CST_W = 1142

F32 = mybir.dt.float32
BF = mybir.dt.bfloat16
U8 = mybir.dt.uint8
OP = mybir.AluOpType
AF = mybir.ActivationFunctionType


def build_nc(stage=5):
    nc = bacc.Bacc()
    embT_d = nc.dram_tensor("embT", [128, G * 129], BF, kind="ExternalInput")
    segT_d = nc.dram_tensor("segT", [128, G * 84], U8, kind="ExternalInput")
    emb4_d = nc.dram_tensor("emb4", [128, NC4], BF, kind="ExternalInput")
    seg4_d = nc.dram_tensor("seg4", [128, NC4], U8, kind="ExternalInput")
    cst_d = nc.dram_tensor("cst", [128, CST_W], BF, kind="ExternalInput")
    xout_d = nc.dram_tensor("xout", [84, 129], F32, kind="ExternalOutput")
    vout_d = nc.dram_tensor("vout", [128, 1], F32, kind="ExternalOutput")

    with ExitStack() as ctx:
        tc = ctx.enter_context(tile.TileContext(nc))
        big = ctx.enter_context(tc.tile_pool(name="big", bufs=1))
        sm = ctx.enter_context(tc.tile_pool(name="sm", bufs=1))
        sqp = ctx.enter_context(tc.tile_pool(name="sqp", bufs=4))
        ps = ctx.enter_context(tc.tile_pool(name="ps", bufs=1, space="PSUM"))
        psD = ctx.enter_context(tc.tile_pool(name="psD", bufs=3, space="PSUM"))

        cst = big.tile([128, CST_W], BF)
        nc.sync.dma_start(out=cst, in_=cst_d[:, :])
        sel32 = big.tile([128, 84], F32)
        nc.vector.tensor_copy(sel32, cst[:, OFF_SEL:OFF_SEL + 84])
        # chunked input DMAs so one-hot builds / pass-1 / pass-2 pipeline
        # against chunk arrivals instead of monolithic loads
        segT = big.tile([128, G * 84], U8)
        for i in range(4):
            w = G * 84 // 4
            nc.sync.dma_start(out=segT[:, i * w:(i + 1) * w],
                              in_=segT_d[:, i * w:(i + 1) * w])
        embT = big.tile([128, G * 129], BF)
        for i in range(8):
            w = G * 129 // 8
            nc.sync.dma_start(out=embT[:, i * w:(i + 1) * w],
                              in_=embT_d[:, i * w:(i + 1) * w])
        seg4 = big.tile([128, NC4], U8)
        for i in range(2):
            w = NC4 // 2
            nc.sync.dma_start(out=seg4[:, i * w:(i + 1) * w],
                              in_=seg4_d[:, i * w:(i + 1) * w])
        emb4 = big.tile([128, NC4], BF)
        for i in range(8):
            w = NC4 // 8
            nc.sync.dma_start(out=emb4[:, i * w:(i + 1) * w],
                              in_=emb4_d[:, i * w:(i + 1) * w])

        # one-hot, pixel-on-partition: ohT[p, g*84 + a*21 + l] = (seg == l)
        ohT = big.tile([128, G * 84], BF)
        for s in range(16):
            sl = slice(s * 672, (s + 1) * 672)
            nc.vector.scalar_tensor_tensor(
                out=ohT[:, sl], in0=segT[:, sl], scalar=0.0,
                in1=cst[:, OFF_IOTA_L:OFF_IOTA_L + 672],
                op0=OP.add, op1=OP.is_equal)

        # ---- pass 1: X[(a,l), (a,d)|counts] = sum_p ohT * embT ----
        X_ps = ps.tile([84, 129], F32)
        for g in range(G):
            nc.tensor.matmul(
                X_ps,
                lhsT=ohT[:, g * 84:(g + 1) * 84],
                rhs=embT[:, g * 129:(g + 1) * 129],
                start=(g == 0), stop=(g == G - 1))
        Xs = sm.tile([84, 129], F32)
        nc.vector.tensor_copy(Xs, X_ps)
        nc.sync.dma_start(out=xout_d[:, :], in_=Xs)

        if stage >= 2:
            # ---- extract sums -> -means (bf16) at 4 partition blocks ----
            M_ps = ps.tile([128, 32], F32)
            C_ps = ps.tile([128, 1], F32)
            for cb in range(4):
                tp = (0, cb * 32)
                for a in range(4):
                    sel = sel32[0:84, a * 21:(a + 1) * 21]
                    nc.tensor.matmul(
                        M_ps[cb * 32:cb * 32 + 21, :], lhsT=sel,
                        rhs=Xs[:, a * 32:(a + 1) * 32],
                        start=(a == 0), stop=(a == 3), tile_position=tp,
                        skip_group_check=True)
                    nc.tensor.matmul(
                        C_ps[cb * 32:cb * 32 + 21, :], lhsT=sel,
                        rhs=Xs[:, 128:129],
                        start=(a == 0), stop=(a == 3), tile_position=tp,
                        skip_group_check=True)

            lhsT_OH = sm.tile([128, 128], BF)
            nc.vector.memset(lhsT_OH, 0.0)
            lhsT_W1 = sm.tile([128, 4], BF)
            nc.vector.memset(lhsT_W1, 0.0)
            lhsT_W8 = sm.tile([128, 256], BF)
            nc.vector.memset(lhsT_W8, 0.0)
            cnt = sm.tile([128, 1], F32)
            rec = sm.tile([128, 1], F32)
            nrec = sm.tile([128, 1], F32)
            pres = sm.tile([128, 1], F32)
            wtmp = sm.tile([128, 1], F32)
            for cb in range(4):
                sl = slice(cb * 32, cb * 32 + 21)
                nc.vector.tensor_scalar(out=cnt[sl], in0=C_ps[sl], scalar1=1.0,
                                        scalar2=None, op0=OP.max)
                nc.vector.reciprocal(rec[sl], cnt[sl])
                nc.vector.tensor_scalar(out=nrec[sl], in0=rec[sl],
                                        scalar1=-1.0, scalar2=None,
                                        op0=OP.mult)
                # lhsT_OH[cb*32+l, cb*32+d] = -sums/cnt = -mean
                nc.vector.scalar_tensor_tensor(
                    out=lhsT_OH[sl, cb * 32:(cb + 1) * 32], in0=M_ps[sl, :],
                    scalar=0.0, in1=nrec[sl].to_broadcast((21, 32)),
                    op0=OP.add, op1=OP.mult)
                nc.vector.tensor_scalar(out=pres[sl], in0=C_ps[sl],
                                        scalar1=0.0, scalar2=None,
                                        op0=OP.is_gt)
                # w = pres * (1/cnt) * fgmask
                nc.vector.scalar_tensor_tensor(
                    out=wtmp[sl], in0=pres[sl], scalar=0.0, in1=rec[sl],
                    op0=OP.add, op1=OP.mult)
                nc.vector.scalar_tensor_tensor(
                    out=lhsT_W1[sl, cb:cb + 1], in0=wtmp[sl], scalar=0.0,
                    in1=cst[sl, OFF_MASK:OFF_MASK + 1],
                    op0=OP.add, op1=OP.mult)
            for u in range(8):
                o = u * 32 + u * 4
                nc.vector.tensor_copy(lhsT_W8[:, o:o + 4], lhsT_W1)

        # one-hot, label-on-partition: oh4[c*32+l, m] = (seg[c*16384+m] == l)
        # (emitted after the extract chain so the tiny critical-path DVE ops
        #  aren't queued behind these big slabs)
        oh4 = big.tile([128, NC4], BF)
        icb = cst[:, OFF_IOTA_COL:OFF_IOTA_COL + 1]
        for s in range(16):
            sl = slice(s * 1024, (s + 1) * 1024)
            nc.vector.scalar_tensor_tensor(
                out=oh4[:, sl], in0=seg4[:, sl], scalar=0.0,
                in1=icb.to_broadcast((128, 1024)),
                op0=OP.add, op1=OP.is_equal)

        if stage >= 3:
            # ---- pass 2 ----
            # ACT-produced bias tiles: keeps every Activation to <=1
            # cross-engine wait (the AC instruction struct has a single
            # sync-wait slot).
            zbias = sm.tile([128, 1], F32)
            nc.scalar.activation(zbias, cst[:, 0:1], AF.Copy, bias=0.0,
                                 scale=0.0)
            nbias2 = sm.tile([128, 1], F32)
            nc.scalar.activation(nbias2, zbias, AF.Copy, bias=-DELTA_V,
                                 scale=0.0)
            A_ps = ps.tile([128, 512], F32)   # per-pixel |e - mu|^2
            B_ps = ps.tile([128, 512], F32)   # per-pixel w
            ident = cst[:, OFF_IDENT:OFF_IDENT + 128]
            for Tt in range(4):
                tp = (0, Tt * 32)
                for u in range(8):
                    t = Tt * 8 + u
                    cols = slice(t * 512, (t + 1) * 512)
                    D_ps = psD.tile([128, 512], F32)
                    nc.tensor.matmul(D_ps, lhsT=ident, rhs=emb4[:, cols],
                                     start=True, stop=False)
                    nc.tensor.matmul(D_ps, lhsT=lhsT_OH, rhs=oh4[:, cols],
                                     start=False, stop=True)
                    sqt = sqp.tile([128, 512], BF)
                    nc.scalar.activation(sqt, D_ps, AF.Square,
                                         bias=zbias[:, 0:1])
                    nc.tensor.matmul(
                        A_ps[Tt * 32:(Tt + 1) * 32, :],
                        lhsT=cst[:, OFF_ONES_BD8 + u * 32:
                                 OFF_ONES_BD8 + (u + 1) * 32],
                        rhs=sqt, start=(u == 0), stop=(u == 7),
                        tile_position=tp, skip_group_check=True)
                    if stage >= 4:
                        nc.tensor.matmul(
                            B_ps[Tt * 32:(Tt + 1) * 32, :],
                            lhsT=lhsT_W8[:, u * 32:(u + 1) * 32],
                            rhs=oh4[:, cols], start=(u == 0), stop=(u == 7),
                            tile_position=tp, skip_group_check=True)

            vn = sm.tile([128, 1], F32)
            # tail: d = sqrt(A); r = max(d - dv, 0); vn = sum(r*r*B)
            d_sb = sm.tile([128, 512], F32)
            nc.scalar.activation(d_sb, A_ps, AF.Sqrt, bias=zbias[:, 0:1])
            r_sb = sm.tile([128, 512], F32)
            nc.vector.tensor_scalar(out=r_sb, in0=d_sb, scalar1=-DELTA_V,
                                    scalar2=0.0, op0=OP.add, op1=OP.max)
            r2_sb = sm.tile([128, 512], F32)
            nc.vector.scalar_tensor_tensor(
                out=r2_sb, in0=r_sb, scalar=0.0, in1=r_sb,
                op0=OP.add, op1=OP.mult)
            vw = sm.tile([128, 512], F32)
            nc.vector.scalar_tensor_tensor(
                out=vw, in0=r2_sb, scalar=0.0, in1=B_ps,
                op0=OP.add, op1=OP.mult, accum_out=vn)
            nc.sync.dma_start(out=vout_d[:, :], in_=vn)
        else:
            vz = sm.tile([128, 1], F32)
            nc.vector.memset(vz, 0.0)
            nc.sync.dma_start(out=vout_d[:, :], in_=vz)

    nc.compile()
    return nc


def _make_consts():
    cst = np.zeros((128, CST_W), np.float32)
    # l pattern per g-block: col a*21+l -> l, tiled for 8-g slabs
    iota_l = np.tile(np.arange(LP), A4)          # [84]
    cst[:, OFF_IOTA_L:OFF_IOTA_L + 672] = np.tile(iota_l, 8)[None, :]
    cst[:, OFF_IOTA_COL] = np.arange(128) % 32
    cst[:, OFF_IDENT:OFF_IDENT + 128] = np.eye(128)
    cst[0:84, OFF_SEL:OFF_SEL + 84] = np.eye(84)
    ones8 = np.zeros((128, 8, 32), np.float32)
    for c in range(C):
        for d in range(32):
            for u in range(8):
                ones8[c * 32 + d, u, u * 4 + c] = 1.0
    cst[:, OFF_ONES_BD8:OFF_ONES_BD8 + 256] = ones8.reshape(128, 256)
    mask = np.zeros(128, np.float32)
    for c in range(C):
        mask[c * 32 + 1:c * 32 + LP] = 1.0
    cst[:, OFF_MASK] = mask
    return cst.astype(BF16)


def _prep_core(emb_b, seg_b, cst):
    """emb_b [32, 65536] f32, seg_b [65536] i32 -> per-core input map."""
    Tm = np.ascontiguousarray(emb_b.T)                       # [N, 32]
    t4 = Tm.reshape(G, 128, A4, 32).transpose(1, 0, 2, 3)    # [p, g, a, d]
    embT = np.empty((128, G, 129), BF16)
    embT[:, :, :128] = t4.reshape(128, G, 128).astype(BF16)
    embT[:, :, 128] = BF16(1.0)
    s4 = seg_b.reshape(G, 128, A4).transpose(1, 0, 2)        # [p, g, a]
    segT = np.ascontiguousarray(
        np.broadcast_to(s4[:, :, :, None], (128, G, A4, LP))
    ).reshape(128, G * 84).astype(np.uint8)
    emb4 = np.ascontiguousarray(
        emb_b.reshape(32, C, NC4).transpose(1, 0, 2)).reshape(128, NC4)
    seg4 = np.ascontiguousarray(
        np.broadcast_to(seg_b.reshape(C, 1, NC4), (C, 32, NC4))
    ).reshape(128, NC4).astype(np.uint8)
    return {
        "embT": embT.reshape(128, G * 129),
        "segT": segT,
        "emb4": emb4.astype(BF16),
        "seg4": seg4,
        "cst": cst,
    }


_NC_CACHE = None


def _get_nc():
    global _NC_CACHE
    if _NC_CACHE is None:
        _NC_CACHE = build_nc()
    return _NC_CACHE


def _host_finish(X, vn):
    """X [84, 129] f32 (pass-1 matrix), vn [128, 1] f32 -> (var_b, dist_b)."""
    Xr = X.reshape(A4, LP, 129).astype(np.float64)
    counts = Xr[:, :, 128].sum(0)                            # [21]
    sums = np.zeros((LP, 32))
    for a in range(A4):
        sums += Xr[a, :, a * 32:(a + 1) * 32]
    means = sums / np.maximum(counts, 1.0)[:, None]
    pres = counts > 0
    pres[0] = False
    nl = float(pres.sum())
    var_b = float(vn.sum()) / max(nl, 1.0) if nl > 0 else 0.0
    m = means[1:]
    p = pres[1:]
    sqd = ((m[:, None, :] - m[None, :, :]) ** 2).sum(-1)
    dist = np.sqrt(np.maximum(sqd, 0.0))
    pair = (p[:, None] & p[None, :]) & ~np.eye(LP - 1, dtype=bool)
    dl = (np.maximum(DELTA_D - dist, 0.0) ** 2 * pair).sum()
    denom = max(nl * (nl - 1.0), 1.0)
    dist_b = dl / denom / 2.0 if nl > 1 else 0.0
    return var_b, dist_b


def kernel(embedding, seg_gt):
    embedding = np.asarray(embedding, np.float32)
    seg_gt = np.asarray(seg_gt, np.int32)
    cst = _make_consts()
    in_maps = [_prep_core(embedding[b], seg_gt[b], cst) for b in range(B)]
    nc = _get_nc()
    res = run_bass_kernel_spmd(nc, in_maps, core_ids=list(range(B)))
    var_l, dist_l = [], []
    for b in range(B):
        var_b, dist_b = _host_finish(res.results[b]["xout"],
                                     res.results[b]["vout"])
        var_l.append(var_b)
        dist_l.append(dist_b)
    return (np.float32(np.mean(var_l)), np.float32(np.mean(dist_l)),
            np.float32(0.0))



# revision 9
# speedup vs baseline: 1.2228x; 1.2228x over previous
"""Trainium2 Bass kernel for DiscriminativeLoss (segment_reduce).

Full inputs: embedding [8, 32, 65536] f32, seg_gt [8, 65536] i32 (labels 0..20,
0 = background).  Output: (var_loss, dist_loss, reg_loss) scalars.

Sharding: pure data parallel — batch b -> core b.  Each core computes, for its
sample:
  pass 1 (pixel-on-partition layout, fp8 embedding): per-label sums+counts
         [84,132] via one-hot matmuls accumulated in PSUM,
  pass 2 (channel-on-partition layout): per-pixel squared distance to own
         centroid via (I | -M) matmuls grouped 4-wide over PSUM banks so the
         ident/-M stationaries are loaded once per group, hinge, and the
         w-weighted global reduction where w_l = present_l / counts_l.
The tiny 21x21 centroid pairwise loss and final scalar assembly run on host
from the per-core [84,132] segment-sum matrix and [128] partial var sums.
"""

import os
import sys
from contextlib import ExitStack

import numpy as np

for _p in ("/opt/trn_rl_repo", "/root/.axon_site/_ro/trn_rl_repo"):
    if os.path.isdir(_p) and _p not in sys.path:
        sys.path.insert(0, _p)

import ml_dtypes

import concourse.bass as bass
import concourse.bacc as bacc
import concourse.tile as tile
from concourse import mybir
from concourse.bass_utils import run_bass_kernel_spmd

BF16 = ml_dtypes.bfloat16
FP8 = ml_dtypes.float8_e4m3

B, D, N = 8, 32, 65536
LP = 21          # label slots 0..20 (0 = background)
C = 4            # chunk count (channel-on-partition packing)
NC4 = N // C     # 16384 pixels per chunk
G = 128          # pass-1 tiles (512 px each)
A4 = 4           # pixels per partition per pass-1 tile
GW = 132         # pass-1 rhs cols per tile: 4 a-blocks of (32 emb + 1 ones)
T2 = 32          # pass-2 tiles (512 cols each)
UG = 4           # pass-2 tiles per PSUM-bank group
DELTA_V = 0.5
DELTA_D = 3.0

EMB4_FP8 = False     # channel-layout embedding in fp8 (extra DMA savings)

# const tensor column offsets (bf16 [128, CST_W])
OFF_IOTA_L = 0            # [128, 672]  l pattern, tiled x8 slabs
OFF_IOTA_COL = 672        # [128, 1]    p % 32
OFF_IDENT = 673           # [128, 128]  identity
OFF_SEL = 801             # [128, 84]   eye(84) selector
OFF_ONES_BD8 = 885        # [128, 256]  8 shifted block-diag ones variants
OFF_MASK = 1141           # [128, 1]    1 for rows c*32+l with 1<=l<=20
CST_W = 1142

F32 = mybir.dt.float32
BF = mybir.dt.bfloat16
F8 = mybir.dt.float8e4
U8 = mybir.dt.uint8
OP = mybir.AluOpType
AF = mybir.ActivationFunctionType

# one-hot build slab split: (engine, slab_idx) lists
OHT_SLABS = 16            # ohT: 16 slabs of 672 cols (8 g each)
OH4_SLABS = 16            # oh4: 16 slabs of 1024 cols
OHT_GPS = set(int(x) for x in os.environ.get('GPS_T', '10,11,12,13,14,15').split(',') if x != '')   # slabs built on gpsimd
OH4_GPS = set(int(x) for x in os.environ.get('GPS_4', '10,11,12,13,14,15').split(',') if x != '')


def build_nc():
    e4dt = F8 if EMB4_FP8 else BF
    nc = bacc.Bacc()
    embT_d = nc.dram_tensor("embT", [128, G * GW], F8, kind="ExternalInput")
    segT_d = nc.dram_tensor("segT", [128, G * 84], U8, kind="ExternalInput")
    emb4_d = nc.dram_tensor("emb4", [128, NC4], e4dt, kind="ExternalInput")
    seg4_d = nc.dram_tensor("seg4", [128, NC4], U8, kind="ExternalInput")
    cst_d = nc.dram_tensor("cst", [128, CST_W], BF, kind="ExternalInput")
    xout_d = nc.dram_tensor("xout", [84, GW], F32, kind="ExternalOutput")
    vout_d = nc.dram_tensor("vout", [128, 1], F32, kind="ExternalOutput")

    with ExitStack() as ctx:
        tc = ctx.enter_context(tile.TileContext(nc))
        big = ctx.enter_context(tc.tile_pool(name="big", bufs=1))
        sm = ctx.enter_context(tc.tile_pool(name="sm", bufs=1))
        sqp = ctx.enter_context(tc.tile_pool(name="sqp", bufs=4))
        ps = ctx.enter_context(tc.tile_pool(name="ps", bufs=1, space="PSUM"))
        psD = ctx.enter_context(tc.tile_pool(name="psD", bufs=1, space="PSUM"))

        cst = big.tile([128, CST_W], BF)
        nc.sync.dma_start(out=cst, in_=cst_d[:, :])

        # warm the ACT table with a Sqrt first so the (only) table set loaded
        # is sqrt_and_others, which also contains square/relu/copy -> no
        # mid-kernel ACT_TABLE_LOAD.  zbias doubles as the all-zero f32 bias.
        zbias = sm.tile([128, 1], F32)
        nc.scalar.activation(zbias, cst[:, 0:1], AF.Sqrt, bias=0.0, scale=0.0)

        sel32 = big.tile([128, 84], F32)
        nc.vector.tensor_copy(sel32, cst[:, OFF_SEL:OFF_SEL + 84])
        icb32 = sm.tile([128, 1], F32)
        nc.vector.tensor_copy(icb32, cst[:, OFF_IOTA_COL:OFF_IOTA_COL + 1])

        # ---- input DMAs, chunked so consumers pipeline against arrivals ----
        segT = big.tile([128, G * 84], U8)
        for i in range(4):
            w = G * 84 // 4
            nc.sync.dma_start(out=segT[:, i * w:(i + 1) * w],
                              in_=segT_d[:, i * w:(i + 1) * w])
        embT = big.tile([128, G * GW], F8)
        for i in range(4):
            w = G * GW // 4
            nc.sync.dma_start(out=embT[:, i * w:(i + 1) * w],
                              in_=embT_d[:, i * w:(i + 1) * w])
        seg4 = big.tile([128, NC4], U8)
        for i in range(2):
            w = NC4 // 2
            nc.sync.dma_start(out=seg4[:, i * w:(i + 1) * w],
                              in_=seg4_d[:, i * w:(i + 1) * w])
        emb4 = big.tile([128, NC4], e4dt)
        for i in range(8):
            w = NC4 // 8
            nc.sync.dma_start(out=emb4[:, i * w:(i + 1) * w],
                              in_=emb4_d[:, i * w:(i + 1) * w])

        # one-hot, pixel-on-partition: ohT[p, g*84 + a*21 + l] = (seg == l)
        # (gpsimd scalar_tensor_tensor fails the V3 ISA opcode check, so all
        #  one-hot builds run on DVE)
        ohT = big.tile([128, G * 84], BF)
        for s in range(OHT_SLABS):
            sl = slice(s * 672, (s + 1) * 672)
            eng = nc.vector
            eng.scalar_tensor_tensor(
                out=ohT[:, sl], in0=segT[:, sl], scalar=0.0,
                in1=cst[:, OFF_IOTA_L:OFF_IOTA_L + 672],
                op0=OP.add, op1=OP.is_equal)

        # ---- pass 1: X[(a,l), (a',(d|1))] = sum_p ohT * embT ----
        X_ps = ps.tile([84, GW], F32)
        for g in range(G):
            nc.tensor.matmul(
                X_ps,
                lhsT=ohT[:, g * 84:(g + 1) * 84],
                rhs=embT[:, g * GW:(g + 1) * GW],
                start=(g == 0), stop=(g == G - 1))
        Xs = sm.tile([84, GW], F32)
        nc.vector.tensor_copy(Xs, X_ps)
        nc.sync.dma_start(out=xout_d[:, :], in_=Xs)

        # ---- extract: sums+counts -> -means (bf16), w (f32), vectorized ----
        # M_ps[cb*32+l, 0:32] = sums, [.,32] = counts (diag-in-a reduction)
        M_ps = ps.tile([128, 33], F32)
        for cb in range(4):
            for a in range(A4):
                nc.tensor.matmul(
                    M_ps[cb * 32:cb * 32 + 21, :],
                    lhsT=sel32[0:84, a * 21:(a + 1) * 21],
                    rhs=Xs[:, a * 33:(a + 1) * 33],
                    start=(a == 0), stop=(a == 3),
                    tile_position=(0, cb * 32), skip_group_check=True)

        cnt = sm.tile([128, 1], F32)
        nc.vector.tensor_scalar(out=cnt, in0=M_ps[:, 32:33], scalar1=1.0,
                                scalar2=None, op0=OP.max)
        rec = sm.tile([128, 1], F32)
        nc.vector.reciprocal(rec, cnt)
        nrec = sm.tile([128, 1], F32)
        nc.vector.tensor_scalar(out=nrec, in0=rec, scalar1=-1.0,
                                scalar2=None, op0=OP.mult)
        pres = sm.tile([128, 1], F32)
        nc.vector.tensor_scalar(out=pres, in0=M_ps[:, 32:33], scalar1=0.0,
                                scalar2=None, op0=OP.is_gt)
        wtmp = sm.tile([128, 1], F32)
        nc.vector.scalar_tensor_tensor(
            out=wtmp, in0=pres, scalar=0.0, in1=rec, op0=OP.add, op1=OP.mult)
        w1 = sm.tile([128, 1], F32)
        nc.vector.scalar_tensor_tensor(
            out=w1, in0=wtmp, scalar=0.0, in1=cst[:, OFF_MASK:OFF_MASK + 1],
            op0=OP.add, op1=OP.mult)
        nmu = sm.tile([128, 32], BF)
        nc.vector.scalar_tensor_tensor(
            out=nmu, in0=M_ps[:, 0:32], scalar=0.0,
            in1=nrec.to_broadcast((128, 32)), op0=OP.add, op1=OP.mult)

        # scatter -mean into block-diagonal stationary
        lhsT_OH = sm.tile([128, 128], BF)
        nc.vector.memset(lhsT_OH, 0.0)
        for cb in range(4):
            sl = slice(cb * 32, cb * 32 + 21)
            nc.vector.tensor_copy(lhsT_OH[sl, cb * 32:(cb + 1) * 32], nmu[sl])
        lhsT_W1 = sm.tile([128, 4], BF)
        nc.vector.memset(lhsT_W1, 0.0)
        for cb in range(4):
            sl = slice(cb * 32, cb * 32 + 21)
            nc.vector.tensor_copy(lhsT_W1[sl, cb:cb + 1], w1[sl])
        lhsT_W8 = sm.tile([128, 256], BF)
        nc.vector.memset(lhsT_W8, 0.0)
        for u in range(8):
            o = u * 32 + u * 4
            nc.vector.tensor_copy(lhsT_W8[:, o:o + 4], lhsT_W1)

        # one-hot, label-on-partition: oh4[c*32+l, m] = (seg[c*16384+m] == l)
        # (emitted after the extract chain so the tiny critical-path DVE ops
        #  aren't queued behind these big slabs)
        # single-src tensor_scalar (per-partition compare target) so the DVE
        # can run a 2-port perf mode instead of 1x scalar_tensor_tensor
        oh4 = big.tile([128, NC4], BF)
        for s in range(OH4_SLABS):
            sl = slice(s * 1024, (s + 1) * 1024)
            nc.vector.tensor_scalar(
                out=oh4[:, sl], in0=seg4[:, sl], scalar1=icb32,
                scalar2=None, op0=OP.is_equal)

        # ---- pass 2, grouped so the 128-col stationaries load once/group ----
        A_ps = ps.tile([128, 512], F32)   # per-pixel |e - mu|^2
        B_ps = ps.tile([128, 512], F32)   # per-pixel w
        ident = cst[:, OFF_IDENT:OFF_IDENT + 128]
        ngrp = T2 // UG
        for grp in range(ngrp):
            banks = [psD.tile([128, 512], F32, name=f"D{u}")
                     for u in range(UG)]
            cols = [slice((grp * UG + u) * 512, (grp * UG + u + 1) * 512)
                    for u in range(UG)]
            for u in range(UG):
                nc.tensor.matmul(banks[u], lhsT=ident, rhs=emb4[:, cols[u]],
                                 start=True, stop=False, skip_group_check=True)
            for u in range(UG):
                nc.tensor.matmul(banks[u], lhsT=lhsT_OH, rhs=oh4[:, cols[u]],
                                 start=False, stop=True, skip_group_check=True)
            for u in range(UG):
                t = grp * UG + u
                Tt, ut = t // 8, t % 8
                tp = (0, Tt * 32)
                sqt = sqp.tile([128, 512], BF)
                nc.scalar.activation(sqt, banks[u], AF.Square,
                                     bias=zbias[:, 0:1])
                nc.tensor.matmul(
                    A_ps[Tt * 32:(Tt + 1) * 32, :],
                    lhsT=cst[:, OFF_ONES_BD8 + ut * 32:
                             OFF_ONES_BD8 + (ut + 1) * 32],
                    rhs=sqt, start=(t % 8 == 0), stop=(t % 8 == 7),
                    tile_position=tp, skip_group_check=True)
                nc.tensor.matmul(
                    B_ps[Tt * 32:(Tt + 1) * 32, :],
                    lhsT=lhsT_W8[:, ut * 32:(ut + 1) * 32],
                    rhs=oh4[:, cols[u]], start=(t % 8 == 0), stop=(t % 8 == 7),
                    tile_position=tp, skip_group_check=True)

        # tail: d = sqrt(A); r = max(d - dv, 0); vn = sum(r*r*B)
        vn = sm.tile([128, 1], F32)
        d_sb = sm.tile([128, 512], F32)
        nc.scalar.activation(d_sb, A_ps, AF.Sqrt, bias=zbias[:, 0:1])
        r_sb = sm.tile([128, 512], F32)
        nc.vector.tensor_scalar(out=r_sb, in0=d_sb, scalar1=-DELTA_V,
                                scalar2=0.0, op0=OP.add, op1=OP.max)
        rw_sb = sm.tile([128, 512], F32)
        nc.vector.scalar_tensor_tensor(
            out=rw_sb, in0=r_sb, scalar=0.0, in1=B_ps,
            op0=OP.add, op1=OP.mult)
        vw = sm.tile([128, 512], F32)
        nc.vector.scalar_tensor_tensor(
            out=vw, in0=rw_sb, scalar=0.0, in1=r_sb,
            op0=OP.add, op1=OP.mult, accum_out=vn)
        nc.sync.dma_start(out=vout_d[:, :], in_=vn)

    nc.compile()
    return nc


def _make_consts():
    cst = np.zeros((128, CST_W), np.float32)
    iota_l = np.tile(np.arange(LP), A4)          # [84]
    cst[:, OFF_IOTA_L:OFF_IOTA_L + 672] = np.tile(iota_l, 8)[None, :]
    cst[:, OFF_IOTA_COL] = np.arange(128) % 32
    cst[:, OFF_IDENT:OFF_IDENT + 128] = np.eye(128)
    cst[0:84, OFF_SEL:OFF_SEL + 84] = np.eye(84)
    ones8 = np.zeros((128, 8, 32), np.float32)
    for c in range(C):
        for d in range(32):
            for u in range(8):
                ones8[c * 32 + d, u, u * 4 + c] = 1.0
    cst[:, OFF_ONES_BD8:OFF_ONES_BD8 + 256] = ones8.reshape(128, 256)
    mask = np.zeros(128, np.float32)
    for c in range(C):
        mask[c * 32 + 1:c * 32 + LP] = 1.0
    cst[:, OFF_MASK] = mask
    return cst.astype(BF16)


def _prep_core(emb_b, seg_b, cst):
    """emb_b [32, 65536] f32, seg_b [65536] i32 -> per-core input map."""
    Tm = np.ascontiguousarray(emb_b.T)                       # [N, 32]
    t4 = Tm.reshape(G, 128, A4, 32).transpose(1, 0, 2, 3)    # [p, g, a, d]
    embT = np.empty((128, G, A4, 33), FP8)
    embT[:, :, :, :32] = t4.astype(FP8)
    embT[:, :, :, 32] = FP8(1.0)
    s4 = seg_b.reshape(G, 128, A4).transpose(1, 0, 2)        # [p, g, a]
    segT = np.ascontiguousarray(
        np.broadcast_to(s4[:, :, :, None], (128, G, A4, LP))
    ).reshape(128, G * 84).astype(np.uint8)
    emb4 = np.ascontiguousarray(
        emb_b.reshape(32, C, NC4).transpose(1, 0, 2)).reshape(128, NC4)
    seg4 = np.ascontiguousarray(
        np.broadcast_to(seg_b.reshape(C, 1, NC4), (C, 32, NC4))
    ).reshape(128, NC4).astype(np.uint8)
    return {
        "embT": embT.reshape(128, G * GW),
        "segT": segT,
        "emb4": emb4.astype(FP8 if EMB4_FP8 else BF16),
        "seg4": seg4,
        "cst": cst,
    }


_NC_CACHE = None


def _get_nc():
    global _NC_CACHE
    if _NC_CACHE is None:
        _NC_CACHE = build_nc()
    return _NC_CACHE


def _host_finish(X, vn):
    """X [84, 132] f32 (pass-1 matrix), vn [128, 1] f32 -> (var_b, dist_b)."""
    Xr = X.reshape(A4, LP, GW).astype(np.float64)
    counts = np.zeros(LP)
    sums = np.zeros((LP, 32))
    for a in range(A4):
        sums += Xr[a, :, a * 33:a * 33 + 32]
        counts += Xr[a, :, a * 33 + 32]
    means = sums / np.maximum(counts, 1.0)[:, None]
    pres = counts > 0
    pres[0] = False
    nl = float(pres.sum())
    var_b = float(vn.sum()) / max(nl, 1.0) if nl > 0 else 0.0
    m = means[1:]
    p = pres[1:]
    sqd = ((m[:, None, :] - m[None, :, :]) ** 2).sum(-1)
    dist = np.sqrt(np.maximum(sqd, 0.0))
    pair = (p[:, None] & p[None, :]) & ~np.eye(LP - 1, dtype=bool)
    dl = (np.maximum(DELTA_D - dist, 0.0) ** 2 * pair).sum()
    denom = max(nl * (nl - 1.0), 1.0)
    dist_b = dl / denom / 2.0 if nl > 1 else 0.0
    return var_b, dist_b


def kernel(embedding, seg_gt):
    embedding = np.asarray(embedding, np.float32)
    seg_gt = np.asarray(seg_gt, np.int32)
    cst = _make_consts()
    in_maps = [_prep_core(embedding[b], seg_gt[b], cst) for b in range(B)]
    nc = _get_nc()
    res = run_bass_kernel_spmd(nc, in_maps, core_ids=list(range(B)))
    var_l, dist_l = [], []
    for b in range(B):
        var_b, dist_b = _host_finish(res.results[b]["xout"],
                                     res.results[b]["vout"])
        var_l.append(var_b)
        dist_l.append(dist_b)
    return (np.float32(np.mean(var_l)), np.float32(np.mean(dist_l)),
            np.float32(0.0))


# revision 12
# speedup vs baseline: 1.3928x; 1.1390x over previous
"""Trainium2 Bass kernel for DiscriminativeLoss (segment_reduce).

Full inputs: embedding [8, 32, 65536] f32, seg_gt [8, 65536] i32 (labels 0..20,
0 = background).  Output: (var_loss, dist_loss, reg_loss) scalars.

Sharding: pure data parallel — batch b -> core b.  Each core computes, for its
sample:
  pass 1 (pixel-on-partition layout, fp8 embedding): per-label sums+counts
         [84,132] via one-hot matmuls accumulated in PSUM,
  pass 2 (channel-on-partition layout): per-pixel squared distance to own
         centroid via (I | -M) matmuls grouped 4-wide over PSUM banks so the
         ident/-M stationaries are loaded once per group, hinge, and the
         w-weighted global reduction where w_l = present_l / counts_l.
The tiny 21x21 centroid pairwise loss and final scalar assembly run on host
from the per-core [84,132] segment-sum matrix and [128] partial var sums.
"""

import os
import sys
from contextlib import ExitStack

import numpy as np

for _p in ("/opt/trn_rl_repo", "/root/.axon_site/_ro/trn_rl_repo"):
    if os.path.isdir(_p) and _p not in sys.path:
        sys.path.insert(0, _p)

import ml_dtypes

import concourse.bass as bass
import concourse.bacc as bacc
import concourse.tile as tile
from concourse import mybir
from concourse.bass_utils import run_bass_kernel_spmd

BF16 = ml_dtypes.bfloat16
FP8 = ml_dtypes.float8_e4m3

B, D, N = 8, 32, 65536
LP = 21          # label slots 0..20 (0 = background)
C = 4            # chunk count (channel-on-partition packing)
NC4 = N // C     # 16384 pixels per chunk
G = 128          # pass-1 tiles (512 px each)
A4 = 4           # pixels per partition per pass-1 tile
GW = 132         # pass-1 rhs cols per tile: 4 a-blocks of (32 emb + 1 ones)
T2 = 32          # pass-2 tiles (512 cols each)
UG = 4           # pass-2 tiles per PSUM-bank group
DELTA_V = 0.5
DELTA_D = 3.0

EMB4_FP8 = False     # channel-layout embedding in fp8 (extra DMA savings)

# const tensor column offsets (bf16 [128, CST_W])
OFF_IOTA_L = 0            # [128, 672]  l pattern, tiled x8 slabs
OFF_IOTA_COL = 672        # [128, 1]    p % 32
OFF_IDENT = 673           # [128, 128]  identity
OFF_SEL = 801             # [128, 84]   eye(84) selector
OFF_ONES_BD8 = 885        # [128, 256]  8 shifted block-diag ones variants
OFF_MASK = 1141           # [128, 1]    1 for rows c*32+l with 1<=l<=20
CST_W = 1142

F32 = mybir.dt.float32
BF = mybir.dt.bfloat16
F8 = mybir.dt.float8e4
U8 = mybir.dt.uint8
OP = mybir.AluOpType
AF = mybir.ActivationFunctionType

# one-hot build slab split: (engine, slab_idx) lists
OHT_SLABS = 16            # ohT: 16 slabs of 672 cols (8 g each)
OH4_SLABS = 16            # oh4: 16 slabs of 1024 cols
OHT_GPS = set(int(x) for x in os.environ.get('GPS_T', '10,11,12,13,14,15').split(',') if x != '')   # slabs built on gpsimd
OH4_GPS = set(int(x) for x in os.environ.get('GPS_4', '10,11,12,13,14,15').split(',') if x != '')


def build_nc():
    e4dt = F8 if EMB4_FP8 else BF
    nc = bacc.Bacc()
    embT_d = nc.dram_tensor("embT", [128, G * GW], F8, kind="ExternalInput")
    segR_d = nc.dram_tensor("segR", [128, G, A4], U8, kind="ExternalInput")
    emb4_d = nc.dram_tensor("emb4", [128, NC4], e4dt, kind="ExternalInput")
    seg4_d = nc.dram_tensor("seg4", [128, NC4], U8, kind="ExternalInput")
    cst_d = nc.dram_tensor("cst", [128, CST_W], BF, kind="ExternalInput")
    xout_d = nc.dram_tensor("xout", [84, GW], F32, kind="ExternalOutput")
    vout_d = nc.dram_tensor("vout", [1, 1], F32, kind="ExternalOutput")

    with ExitStack() as ctx:
        tc = ctx.enter_context(tile.TileContext(nc))
        big = ctx.enter_context(tc.tile_pool(name="big", bufs=1))
        sm = ctx.enter_context(tc.tile_pool(name="sm", bufs=1))
        sqp = ctx.enter_context(tc.tile_pool(name="sqp", bufs=4))
        ps = ctx.enter_context(tc.tile_pool(name="ps", bufs=1, space="PSUM"))
        psD = ctx.enter_context(tc.tile_pool(name="psD", bufs=1, space="PSUM"))

        cst = big.tile([128, CST_W], BF)
        nc.sync.dma_start(out=cst, in_=cst_d[:, :])

        # warm the ACT table with a Sqrt first so the (only) table set loaded
        # is sqrt_and_others, which also contains square/relu/copy -> no
        # mid-kernel ACT_TABLE_LOAD.  zbias doubles as the all-zero f32 bias.
        zbias = sm.tile([128, 1], F32)
        nc.scalar.activation(zbias, cst[:, 0:1], AF.Sqrt, bias=0.0, scale=0.0)

        sel32 = big.tile([128, 84], F32)
        nc.vector.tensor_copy(sel32, cst[:, OFF_SEL:OFF_SEL + 84])
        icb32 = sm.tile([128, 1], F32)
        nc.vector.tensor_copy(icb32, cst[:, OFF_IOTA_COL:OFF_IOTA_COL + 1])

        # ---- input DMAs, chunked so consumers pipeline against arrivals ----
        segR = big.tile([128, G, A4], U8)
        nc.sync.dma_start(out=segR, in_=segR_d[:, :, :])
        embT = big.tile([128, G * GW], F8)
        for i in range(4):
            w = G * GW // 4
            nc.sync.dma_start(out=embT[:, i * w:(i + 1) * w],
                              in_=embT_d[:, i * w:(i + 1) * w])
        # pass-2 feeds: emb4 chunk pairs with the matching seg4 chunk behind
        seg4 = big.tile([128, NC4], U8)
        emb4 = big.tile([128, NC4], e4dt)
        we, ws = NC4 // 8, NC4 // 4
        for j in range(4):
            nc.sync.dma_start(out=emb4[:, 2 * j * we:(2 * j + 1) * we],
                              in_=emb4_d[:, 2 * j * we:(2 * j + 1) * we])
            nc.sync.dma_start(out=emb4[:, (2 * j + 1) * we:(2 * j + 2) * we],
                              in_=emb4_d[:, (2 * j + 1) * we:(2 * j + 2) * we])
            nc.sync.dma_start(out=seg4[:, j * ws:(j + 1) * ws],
                              in_=seg4_d[:, j * ws:(j + 1) * ws])

        # stationaries for the extract scatter, zeroed off the critical path
        lhsT_OH = sm.tile([128, 128], BF)
        nc.vector.memset(lhsT_OH, 0.0)
        lhsT_W1 = sm.tile([128, 4], BF)
        nc.vector.memset(lhsT_W1, 0.0)
        lhsT_W8 = sm.tile([128, 256], BF)
        nc.vector.memset(lhsT_W8, 0.0)
        ones1 = sm.tile([128, 1], F32)
        nc.vector.memset(ones1, 1.0)

        # one-hot, pixel-on-partition: ohT[p, g, l*4+a] = (seg[p,g,a] == l)
        # 21 single-src tensor_scalar ops (immediate compare target) so the
        # DVE can run a 2-port perf mode; strided output keeps each per-g
        # lhsT slice contiguous with a single free dim for LDWEIGHTS
        ohT = big.tile([128, G, 84], BF)
        for l in range(LP):
            nc.vector.tensor_scalar(
                out=ohT[:, :, l * A4:(l + 1) * A4], in0=segR,
                scalar1=float(l), scalar2=None, op0=OP.is_equal)

        # ---- pass 1: X[(a,l), (a',(d|1))] = sum_p ohT * embT ----
        X_ps = ps.tile([84, GW], F32)
        for g in range(G):
            nc.tensor.matmul(
                X_ps,
                lhsT=ohT[:, g, :],
                rhs=embT[:, g * GW:(g + 1) * GW],
                start=(g == 0), stop=(g == G - 1))
        Xs = sm.tile([84, GW], F32)
        nc.vector.tensor_copy(Xs, X_ps)
        nc.sync.dma_start(out=xout_d[:, :], in_=Xs)

        # ---- extract: sums+counts -> -means (bf16), w (f32), vectorized ----
        # M_ps[cb*32+l, 0:32] = sums, [.,32] = counts (diag-in-a reduction)
        M_ps = ps.tile([128, 33], F32)
        for cb in range(4):
            for a in range(A4):
                nc.tensor.matmul(
                    M_ps[cb * 32:cb * 32 + 21, :],
                    lhsT=sel32[0:84, a * 21:(a + 1) * 21],
                    rhs=Xs[:, a * 33:(a + 1) * 33],
                    start=(a == 0), stop=(a == 3),
                    tile_position=(0, cb * 32), skip_group_check=True)

        with tc.high_priority():
            cnt = sm.tile([128, 1], F32)
            nc.vector.tensor_scalar(out=cnt, in0=M_ps[:, 32:33], scalar1=1.0,
                                    scalar2=None, op0=OP.max)
            rec = sm.tile([128, 1], F32)
            nc.vector.reciprocal(rec, cnt)
            pres = sm.tile([128, 1], F32)
            nc.vector.tensor_scalar(out=pres, in0=M_ps[:, 32:33], scalar1=0.0,
                                    scalar2=None, op0=OP.is_gt)
            # w = pres * mask * (1/cnt)
            w1 = sm.tile([128, 1], F32)
            nc.vector.scalar_tensor_tensor(
                out=w1, in0=pres, scalar=cst[:, OFF_MASK:OFF_MASK + 1],
                in1=rec, op0=OP.mult, op1=OP.mult)
            # -mean = sums * (-1) * (1/cnt)
            nmu = sm.tile([128, 32], BF)
            nc.vector.scalar_tensor_tensor(
                out=nmu, in0=M_ps[:, 0:32], scalar=-1.0,
                in1=rec.to_broadcast((128, 32)), op0=OP.mult, op1=OP.mult)
            for cb in range(4):
                sl = slice(cb * 32, cb * 32 + 21)
                nc.vector.tensor_copy(lhsT_OH[sl, cb * 32:(cb + 1) * 32],
                                      nmu[sl])
                nc.vector.tensor_copy(lhsT_W1[sl, cb:cb + 1], w1[sl])
            for u in range(8):
                o = u * 32 + u * 4
                nc.vector.tensor_copy(lhsT_W8[:, o:o + 4], lhsT_W1)

        # one-hot, label-on-partition: oh4[c*32+l, m] = (seg[c*16384+m] == l)
        # (emitted after the extract chain so the tiny critical-path DVE ops
        #  aren't queued behind these big slabs)
        # single-src tensor_scalar (per-partition compare target) so the DVE
        # can run a 2-port perf mode instead of 1x scalar_tensor_tensor
        oh4 = big.tile([128, NC4], BF)
        for s in range(OH4_SLABS):
            sl = slice(s * 1024, (s + 1) * 1024)
            nc.vector.tensor_scalar(
                out=oh4[:, sl], in0=seg4[:, sl], scalar1=icb32,
                scalar2=None, op0=OP.is_equal)

        # ---- pass 2, grouped so the 128-col stationaries load once/group ----
        A_ps = ps.tile([128, 512], F32)   # per-pixel |e - mu|^2
        B_ps = ps.tile([128, 512], F32)   # per-pixel w
        ident = cst[:, OFF_IDENT:OFF_IDENT + 128]
        ngrp = T2 // UG
        for grp in range(ngrp):
            banks = [psD.tile([128, 512], F32, name=f"D{u}")
                     for u in range(UG)]
            cols = [slice((grp * UG + u) * 512, (grp * UG + u + 1) * 512)
                    for u in range(UG)]
            for u in range(UG):
                nc.tensor.matmul(banks[u], lhsT=ident, rhs=emb4[:, cols[u]],
                                 start=True, stop=False, skip_group_check=True)
            for u in range(UG):
                nc.tensor.matmul(banks[u], lhsT=lhsT_OH, rhs=oh4[:, cols[u]],
                                 start=False, stop=True, skip_group_check=True)
            for u in range(UG):
                t = grp * UG + u
                Tt, ut = t // 8, t % 8
                tp = (0, Tt * 32)
                sqt = sqp.tile([128, 512], BF)
                nc.scalar.activation(sqt, banks[u], AF.Square,
                                     bias=zbias[:, 0:1])
                nc.tensor.matmul(
                    A_ps[Tt * 32:(Tt + 1) * 32, :],
                    lhsT=cst[:, OFF_ONES_BD8 + ut * 32:
                             OFF_ONES_BD8 + (ut + 1) * 32],
                    rhs=sqt, start=(t % 8 == 0), stop=(t % 8 == 7),
                    tile_position=tp, skip_group_check=True)
                nc.tensor.matmul(
                    B_ps[Tt * 32:(Tt + 1) * 32, :],
                    lhsT=lhsT_W8[:, ut * 32:(ut + 1) * 32],
                    rhs=oh4[:, cols[u]], start=(t % 8 == 0), stop=(t % 8 == 7),
                    tile_position=tp, skip_group_check=True)

        # tail: d = sqrt(A); r = max(d - dv, 0); vn = sum(r*r*B)
        vn = sm.tile([128, 1], F32)
        d_sb = sm.tile([128, 512], F32)
        nc.scalar.activation(d_sb, A_ps, AF.Sqrt, bias=zbias[:, 0:1])
        r_sb = sm.tile([128, 512], F32)
        nc.vector.tensor_scalar(out=r_sb, in0=d_sb, scalar1=-DELTA_V,
                                scalar2=0.0, op0=OP.add, op1=OP.max)
        rw_sb = sm.tile([128, 512], F32)
        nc.vector.scalar_tensor_tensor(
            out=rw_sb, in0=r_sb, scalar=0.0, in1=B_ps,
            op0=OP.add, op1=OP.mult)
        vw = sm.tile([128, 512], F32)
        nc.vector.scalar_tensor_tensor(
            out=vw, in0=rw_sb, scalar=0.0, in1=r_sb,
            op0=OP.add, op1=OP.mult, accum_out=vn)
        # reduce the per-partition partials to one scalar so the final DMA
        # is a single-descriptor 4-byte write (16-engine sem-inc tail cost)
        nc.tensor.matmul(M_ps[0:1, 0:1], lhsT=ones1, rhs=vn,
                         start=True, stop=True, skip_group_check=True)
        vs_sb = sm.tile([1, 1], F32)
        nc.vector.tensor_copy(vs_sb, M_ps[0:1, 0:1])
        nc.sync.dma_start(out=vout_d[:, :], in_=vs_sb)

    nc.compile()
    return nc


def _make_consts():
    cst = np.zeros((128, CST_W), np.float32)
    iota_l = np.tile(np.arange(LP), A4)          # [84]
    cst[:, OFF_IOTA_L:OFF_IOTA_L + 672] = np.tile(iota_l, 8)[None, :]
    cst[:, OFF_IOTA_COL] = np.arange(128) % 32
    cst[:, OFF_IDENT:OFF_IDENT + 128] = np.eye(128)
    sel = np.zeros((84, 84), np.float32)     # rows (l,a)=l*4+a, col a*21+l
    for l in range(LP):
        for a in range(A4):
            sel[l * A4 + a, a * LP + l] = 1.0
    cst[0:84, OFF_SEL:OFF_SEL + 84] = sel
    ones8 = np.zeros((128, 8, 32), np.float32)
    for c in range(C):
        for d in range(32):
            for u in range(8):
                ones8[c * 32 + d, u, u * 4 + c] = 1.0
    cst[:, OFF_ONES_BD8:OFF_ONES_BD8 + 256] = ones8.reshape(128, 256)
    mask = np.zeros(128, np.float32)
    for c in range(C):
        mask[c * 32 + 1:c * 32 + LP] = 1.0
    cst[:, OFF_MASK] = mask
    return cst.astype(BF16)


def _prep_core(emb_b, seg_b, cst):
    """emb_b [32, 65536] f32, seg_b [65536] i32 -> per-core input map."""
    Tm = np.ascontiguousarray(emb_b.T)                       # [N, 32]
    t4 = Tm.reshape(G, 128, A4, 32).transpose(1, 0, 2, 3)    # [p, g, a, d]
    embT = np.empty((128, G, A4, 33), FP8)
    embT[:, :, :, :32] = t4.astype(FP8)
    embT[:, :, :, 32] = FP8(1.0)
    s4 = seg_b.reshape(G, 128, A4).transpose(1, 0, 2)        # [p, g, a]
    segR = np.ascontiguousarray(s4).reshape(128, G, A4).astype(np.uint8)
    emb4 = np.ascontiguousarray(
        emb_b.reshape(32, C, NC4).transpose(1, 0, 2)).reshape(128, NC4)
    seg4 = np.ascontiguousarray(
        np.broadcast_to(seg_b.reshape(C, 1, NC4), (C, 32, NC4))
    ).reshape(128, NC4).astype(np.uint8)
    return {
        "embT": embT.reshape(128, G * GW),
        "segR": segR,
        "emb4": emb4.astype(FP8 if EMB4_FP8 else BF16),
        "seg4": seg4,
        "cst": cst,
    }


_NC_CACHE = None


def _get_nc():
    global _NC_CACHE
    if _NC_CACHE is None:
        _NC_CACHE = build_nc()
    return _NC_CACHE


def _host_finish(X, vn):
    """X [84, 132] f32 (pass-1 matrix), vn [128, 1] f32 -> (var_b, dist_b)."""
    Xr = X.reshape(LP, A4, GW).astype(np.float64)
    counts = np.zeros(LP)
    sums = np.zeros((LP, 32))
    for a in range(A4):
        sums += Xr[:, a, a * 33:a * 33 + 32]
        counts += Xr[:, a, a * 33 + 32]
    means = sums / np.maximum(counts, 1.0)[:, None]
    pres = counts > 0
    pres[0] = False
    nl = float(pres.sum())
    var_b = float(vn.sum()) / max(nl, 1.0) if nl > 0 else 0.0
    m = means[1:]
    p = pres[1:]
    sqd = ((m[:, None, :] - m[None, :, :]) ** 2).sum(-1)
    dist = np.sqrt(np.maximum(sqd, 0.0))
    pair = (p[:, None] & p[None, :]) & ~np.eye(LP - 1, dtype=bool)
    dl = (np.maximum(DELTA_D - dist, 0.0) ** 2 * pair).sum()
    denom = max(nl * (nl - 1.0), 1.0)
    dist_b = dl / denom / 2.0 if nl > 1 else 0.0
    return var_b, dist_b


def kernel(embedding, seg_gt):
    embedding = np.asarray(embedding, np.float32)
    seg_gt = np.asarray(seg_gt, np.int32)
    cst = _make_consts()
    in_maps = [_prep_core(embedding[b], seg_gt[b], cst) for b in range(B)]
    nc = _get_nc()
    res = run_bass_kernel_spmd(nc, in_maps, core_ids=list(range(B)))
    var_l, dist_l = [], []
    for b in range(B):
        var_b, dist_b = _host_finish(res.results[b]["xout"],
                                     res.results[b]["vout"])
        var_l.append(var_b)
        dist_l.append(dist_b)
    return (np.float32(np.mean(var_l)), np.float32(np.mean(dist_l)),
            np.float32(0.0))
